# revision 2
# baseline (speedup 1.0000x reference)
"""Trainium2 Bass kernel for nn_C3_layer (dense 5x5 VALID conv, 6->16 ch).

Full input x [32,6,512,512] f32 -> full output [32,16,508,508] f32.
Data-parallel over batch: 4 images per core across 8 NeuronCores.

v2 design (f=2 kw-folding; PE floor 216us vs v1's 271us):
  - bf16 end-to-end on device (norm-wise quantization error ~2.7e-3 << 2e-2).
  - Host prepares xd[b, d, ci, h, w]: d=0 is x, d=1 is x shifted one column
    left (free on host).  The moving tile for a 6-row output block is
    [120, 512]: partitions p = d*60 + i*6 + ci over 10 input rows, both
    dup-planes.  A matmul column offset `off` then reads tap kw=off on the
    d=0 plane and kw=off+1 on the d=1 plane simultaneously, so THREE
    matmuls with offsets {0,2,3} cover all 5 taps: {0,1}, {2,3}, {4 on d=1
    only; d=0 stationary rows zero}.  3 passes / 6 rows = 0.5 passes/row
    vs v1's 5/8 = 0.625 -> PE cycles drop 20%.
  - Per block: 3 matmuls (K=120, M=96=(co,r), N=508) accumulate into one
    PSUM half-tile; block pairs share a [96,1024] 2-bank psum tile and are
    evacuated (+bias, f32->bf16) by ONE strided instruction alternating
    DVE/ACT.
  - One in-DMA per block ([120,512] from the host-duplicated xd; 1KB
    descriptors).  In-DMA issue round-robins over engines (DMA issue costs
    ~565ns SEQ time each on SP; spreading avoids a sequencer bottleneck).
  - Out tiles hold 6 blocks [96, 6*508]; device y stays in tile layout,
    host reorders (free).
  - CONV_DYN_LOOP=1 adds a runtime trip count used by bench.py's slope
    timing: one NEFF serves every L.
"""

import os

import numpy as np

KK = 5
R = 6                 # output rows per full block
B_PER_CORE = 4
N_CORES = 8
H = 512
W = 512
HO = H - 4
WO = W - 4
NFULL = 84            # full 6-row blocks per image (504 rows)
TILES = 14            # out tiles per image, 6 blocks each
TAIL_R = 4            # tail block rows (504..507)
OFFS = (0, 2, 3)      # matmul column offsets (pass p covers taps off, off+1)

CH3 = np.array([[0, 1, 2], [1, 2, 3], [2, 3, 4], [3, 4, 5], [0, 4, 5], [0, 1, 5]])
CH4 = np.array([[0, 1, 2, 3], [1, 2, 3, 4], [2, 3, 4, 5], [0, 3, 4, 5], [0, 1, 4, 5],
                [0, 1, 2, 5], [0, 1, 3, 4], [1, 2, 4, 5], [0, 2, 3, 5]])

LAST_RESULTS = None


def _tap(p, d):
    """Which kw tap pass p, dup-plane d covers (None = stationary zero)."""
    return {(0, 0): 0, (0, 1): 1, (1, 0): 2, (1, 1): 3,
            (2, 0): None, (2, 1): 4}[(p, d)]


def _build_full_kernel(w3, w4, w6):
    Wf = np.zeros((16, 6, KK, KK), dtype=np.float32)
    Wf[np.arange(6)[:, None], CH3] = w3
    Wf[(6 + np.arange(9))[:, None], CH4] = w4
    Wf[15] = w6[0]
    return Wf


def _build_stationaries(Wf):
    """TS [3, 120, 96] full-block and TT [3, 96, 64] tail stationaries.

    TS[p][d*60 + i*6 + ci, co*6 + r] = Wf[co, ci, i-r, tap(p,d)]
    TT[p][d*48 + i*6 + ci, co*4 + r] = Wf[co, ci, i-r, tap(p,d)]
    """
    def build(rows, r_blk):
        T = np.zeros((3, 2 * rows * 6, 16 * r_blk), dtype=np.float32)
        for p in range(3):
            for d in range(2):
                t = _tap(p, d)
                if t is None:
                    continue
                for r in range(r_blk):
                    for kh in range(KK):
                        i = r + kh
                        if i >= rows:
                            continue
                        for ci in range(6):
                            T[p, d * rows * 6 + i * 6 + ci,
                              np.arange(16) * r_blk + r] = Wf[:, ci, kh, t]
        return T

    return build(10, R), build(8, TAIL_R)


def _build_bass():
    import contextlib

    import concourse.bacc as bacc
    import concourse.mybir as mybir
    import concourse.tile as tile

    f32 = mybir.dt.float32
    bf16 = mybir.dt.bfloat16
    loop_n = int(os.environ.get("CONV_BENCH_LOOP", "1"))
    dyn_loop = bool(int(os.environ.get("CONV_DYN_LOOP", "0")))

    nc = bacc.Bacc(name="conv5x5v5")
    xd = nc.dram_tensor("xd", [B_PER_CORE, 2, 6, H, W], bf16,
                        kind="ExternalInput")
    loopn = (nc.dram_tensor("loopn", [1, 1], mybir.dt.uint32,
                            kind="ExternalInput") if dyn_loop else None)
    ts = nc.dram_tensor("ts", [3, 120, 96], bf16, kind="ExternalInput")
    tt = nc.dram_tensor("tt", [3, 96, 64], bf16, kind="ExternalInput")
    bias6 = nc.dram_tensor("bias6", [96, 1], f32, kind="ExternalInput")
    bias4 = nc.dram_tensor("bias4", [64, 1], f32, kind="ExternalInput")
    # device out stays in tile layout: y[b, t, co*6+r, u*508+w] (block 6t+u),
    # tail yt[b, co*4+r, w].  Host reorders to [b, co, h, w] (free).
    y = nc.dram_tensor("y", [B_PER_CORE, TILES, 96, 6 * WO], bf16,
                       kind="ExternalOutput")
    yt = nc.dram_tensor("yt", [B_PER_CORE, 64, WO], bf16,
                        kind="ExternalOutput")

    with tile.TileContext(nc) as tc:
        with (
            tc.tile_pool(name="const", bufs=1) as const_pool,
            tc.tile_pool(name="xin",
                         bufs=int(os.environ.get("CONV_IN_BUFS", "6"))) as in_pool,
            tc.tile_pool(name="yout",
                         bufs=int(os.environ.get("CONV_OUT_BUFS", "3"))) as out_pool,
            tc.tile_pool(name="psum",
                         bufs=int(os.environ.get("CONV_PSUM_BUFS", "4")),
                         space="PSUM") as psum_pool,
        ):
            ts_sb = const_pool.tile([120, 3 * 96], bf16, name="ts_sb")
            nc.sync.dma_start(out=ts_sb[:, :], in_=ts.rearrange("p k m -> k p m"))
            tt_sb = const_pool.tile([96, 3 * 64], bf16, name="tt_sb")
            nc.sync.dma_start(out=tt_sb[:, :], in_=tt.rearrange("p k m -> k p m"))
            bias6_sb = const_pool.tile([96, 1], f32, name="bias6_sb")
            nc.sync.dma_start(out=bias6_sb[:, :], in_=bias6[:, :])
            bias4_sb = const_pool.tile([64, 1], f32, name="bias4_sb")
            nc.sync.dma_start(out=bias4_sb[:, :], in_=bias4[:, :])

            if dyn_loop:
                ln_sb = const_pool.tile([1, 1], mybir.dt.uint32, name="ln_sb")
                nc.sync.dma_start(out=ln_sb[:, :], in_=loopn[:, :])
                ln = nc.values_load(ln_sb[0:1, 0:1], min_val=0,
                                    max_val=1 << 20,
                                    skip_runtime_bounds_check=True)
                loop_cm = tc.For_i(0, ln, 1)
            else:
                loop_cm = (tc.For_i(0, loop_n, 1) if loop_n > 1
                           else contextlib.nullcontext())
            with loop_cm:
                _emit_body(nc, mybir, xd, y, yt, ts_sb, tt_sb,
                           bias6_sb, bias4_sb, in_pool, out_pool, psum_pool,
                           f32, bf16)
    nc.finalize()
    return nc


def _emit_body(nc, mybir, xd, y, yt, ts_sb, tt_sb, bias6_sb, bias4_sb,
               in_pool, out_pool, psum_pool, f32, bf16):
    Ident = mybir.ActivationFunctionType.Identity
    # sim-probe switches (leave at defaults for real runs)
    skip_in = bool(int(os.environ.get("CONV_SKIP_IN", "0")))
    skip_out = bool(int(os.environ.get("CONV_SKIP_OUT", "0")))
    skip_evac = bool(int(os.environ.get("CONV_SKIP_EVAC", "0")))
    skip_mm = bool(int(os.environ.get("CONV_SKIP_MM", "0")))

    in_engs = os.environ.get("CONV_IN_ENG", "sync,gpsimd").split(",")
    in_ctr = [0]

    def in_dma(*a, **k):
        if not skip_in:
            eng = in_engs[in_ctr[0] % len(in_engs)]
            in_ctr[0] += 1
            getattr(nc, eng).dma_start(*a, **k)

    out_eng = os.environ.get("CONV_OUT_ENG", "sync")

    def out_dma(*a, **k):
        if not skip_out:
            getattr(nc, out_eng).dma_start(*a, **k)

    def mm(*a, **k):
        if not skip_mm:
            nc.tensor.matmul(*a, **k)

    evac_ctr = [0]

    def evac(ot_view, ps_view, b_ap):
        if skip_evac:
            return
        if evac_ctr[0] % 2 == 0:
            nc.vector.tensor_scalar_add(ot_view, ps_view, b_ap)
        else:
            nc.scalar.activation(ot_view, ps_view, Ident, bias=b_ap,
                                 scale=1.0)
        evac_ctr[0] += 1

    for b in range(B_PER_CORE):
        for t in range(TILES):
            ot = out_pool.tile([96, 6 * WO], bf16, name="ot", tag="ot")
            for s in range(3):
                ps = psum_pool.tile([96, 1024], f32, name="ps", tag="ps")
                for j in range(2):
                    blk = 6 * t + 2 * s + j
                    h0 = R * blk
                    xin = in_pool.tile([120, W], bf16, name="xin", tag="xin")
                    in_dma(
                        out=xin[:, :],
                        in_=xd[b, :, :, h0:h0 + 10, :].rearrange(
                            "d c h w -> (d h c) w"),
                    )
                    for p, off in enumerate(OFFS):
                        mm(
                            ps[:, j * 512:j * 512 + WO],
                            ts_sb[:, p * 96:(p + 1) * 96],
                            xin[:, off:off + WO],
                            start=(p == 0),
                            stop=(p == 2),
                        )
                # one evac instr per block pair (strided 2x508 view)
                ps_v = ps[:, :].rearrange("p (j w) -> p j w", j=2)[:, :, 0:WO]
                ot_v = ot[:, 2 * s * WO:(2 * s + 2) * WO].rearrange(
                    "p (j w) -> p j w", j=2)
                evac(ot_v, ps_v, bias6_sb[:, :])
            out_dma(out=y[b, t], in_=ot[:, :])

        # tail block: output rows 504..507, input rows 504..511
        xin = in_pool.tile([96, W], bf16, name="xin", tag="xin")
        in_dma(
            out=xin[:, :],
            in_=xd[b, :, :, 504:512, :].rearrange("d c h w -> (d h c) w"),
        )
        ps = psum_pool.tile([64, 512], f32, name="pst", tag="pst")
        for p, off in enumerate(OFFS):
            mm(
                ps[:, 0:WO],
                tt_sb[:, p * 64:(p + 1) * 64],
                xin[:, off:off + WO],
                start=(p == 0),
                stop=(p == 2),
            )
        ott = out_pool.tile([64, WO], bf16, name="ott", tag="ott")
        evac(ott[:, :], ps[:, 0:WO], bias4_sb[:, :])
        out_dma(out=yt[b], in_=ott[:, :])


def build_in_maps(x, w3, b3, w4, b4, w6, b6):
    import ml_dtypes

    bf = ml_dtypes.bfloat16
    x = np.asarray(x, dtype=np.float32)
    Wf = _build_full_kernel(np.asarray(w3, dtype=np.float32),
                            np.asarray(w4, dtype=np.float32),
                            np.asarray(w6, dtype=np.float32))
    TS, TT = _build_stationaries(Wf)
    bias16 = np.concatenate([np.asarray(b3, dtype=np.float32),
                             np.asarray(b4, dtype=np.float32),
                             np.asarray(b6, dtype=np.float32)])
    bias6_col = np.ascontiguousarray(np.repeat(bias16, 6)[:, None],
                                     dtype=np.float32)
    bias4_col = np.ascontiguousarray(np.repeat(bias16, 4)[:, None],
                                     dtype=np.float32)
    xbf = x.astype(bf)
    # xd[b, d, ci, h, w]: d=0 -> x, d=1 -> x shifted one column left
    xd = np.zeros((x.shape[0], 2, 6, H, W), dtype=bf)
    xd[:, 0] = xbf
    xd[:, 1, :, :, :W - 1] = xbf[:, :, :, 1:]
    maps = [
        {"xd": np.ascontiguousarray(xd[i * B_PER_CORE:(i + 1) * B_PER_CORE]),
         "ts": TS.astype(bf), "tt": TT.astype(bf),
         "bias6": bias6_col, "bias4": bias4_col}
        for i in range(N_CORES)
    ]
    if bool(int(os.environ.get("CONV_DYN_LOOP", "0"))):
        for m in maps:
            m["loopn"] = np.array([[1]], dtype=np.uint32)
    return maps


def kernel(x, w3, b3, w4, b4, w6, b6):
    global LAST_RESULTS
    from concourse.bass_utils import run_bass_kernel_spmd

    in_maps = build_in_maps(x, w3, b3, w4, b4, w6, b6)
    nc = _build_bass()
    res = run_bass_kernel_spmd(
        nc, in_maps, core_ids=list(range(N_CORES)),
        trace=bool(int(os.environ.get("CONV_TRACE", "0"))),
    )
    LAST_RESULTS = res
    return np.concatenate(
        [unpack_y(r["y"], r["yt"]) for r in res.results], axis=0)


def unpack_y(y_dev, yt_dev):
    """y [B,14,96,3048] + yt [B,64,508] bf16 tile layout -> [B,16,508,508] f32."""
    y_dev = np.asarray(y_dev).astype(np.float32)
    yt_dev = np.asarray(yt_dev).astype(np.float32)
    out = np.empty((B_PER_CORE, 16, HO, WO), dtype=np.float32)
    # y[b, t, co*6+r, u*508+w] -> out row 6*(6t+u)+r
    v = y_dev.reshape(B_PER_CORE, TILES, 16, R, 6, WO)
    out[:, :, :6 * NFULL] = v.transpose(0, 2, 1, 4, 3, 5).reshape(
        B_PER_CORE, 16, 6 * NFULL, WO)
    out[:, :, 6 * NFULL:] = yt_dev.reshape(B_PER_CORE, 16, TAIL_R, WO)
    return out
